# revision 15
# baseline (speedup 1.0000x reference)
"""Trainium2 Bass kernel for nn_CachePredictor (moe_routing).

Computation (see reference):
    x = relu(feature @ W_up.T + b_up)                      [B, 512]
    t_out = sigmoid(einsum('bf,bgf', x, W_table[tids]) + b_table[tids]) * tmask
    i_out = sigmoid(einsum('bf,bgf', x, W_index[iids]) + b_index[iids]) * imask
    out = stack([t_out, i_out])                            [2, B, 256]

Strategy: expert sharding. Per-sample gather of expert weights would move
~4 GB of HBM traffic; grouping samples by expert reads each expert matrix
exactly once (~96 MiB aggregate). Each of the 8 cores owns 8 table experts
and 16 index experts (~12.6 MiB of weights) and processes only the samples
routed to its experts. The host computes routing metadata (sample->expert
grouping, capacity padding) and arranges per-core inputs; all FLOPs (both
matmul stages, relu, sigmoid, bias adds) run on device.

Device program per core (single SPMD program; capacity-padded so all cores
share identical shapes):
  stage 1:  xT[512, nCols] = relu(W_upT.T @ featT + b_up)   PE + ACT(bias)
  stage 2:  per expert PAIR: out[rows, 2*256] = sigmoid(xT_seg.T @ [W_e0|W_e1].T + b)
Matmul operands are float32r (TF32-like 12-bit-mantissa streaming; full PE
rate at moving dim >= 256; ~2e-4 rel err). Experts are processed in PAIRS
sharing one column segment (the union of both experts' samples): every
sample in the segment is multiplied against BOTH experts' weights with one
N=512 moving pass, and the host keeps the valid half. This halves PE
instruction count (the per-instruction LDWEIGHTS + fixed overhead, ~350 ns,
dominates PE time otherwise) at the cost of cheap redundant FLOPs. The
pair's bias add is a K=1 ones-outer-product matmul issued FIRST so it
initializes the full PSUM zero region. Expert weights are host-packed
partition-major into 2 MiB chunks of 4 experts so each weight load is one
large fully-contiguous DMA (>=340 GB/s regime). Activations stay
transposed ([feature, sample]) end to end.

Masked-off samples are never routed (reference zeroes them); the host
scatters computed rows back and leaves the rest zero.
"""

import ml_dtypes
import numpy as np

_N_CORES = 8
_F = 256        # feature dim
_HID = 512      # up-projection width
_G = 256        # buckets
_N_TABLES = 64
_N_INDEXES = 128
_TPC = _N_TABLES // _N_CORES    # table experts per core
_IPC = _N_INDEXES // _N_CORES   # index experts per core
_CPE = 4                        # experts per weight chunk (2 MiB)
_TPAIRS = _N_TABLES // 2        # global table pairs
_IPAIRS = _N_INDEXES // 2       # global index pairs

_nc_cache = {}

# Set by a test harness to capture HW profiles; harmless when unused.
TRACE = False
LAST_RESULTS = None


def _build(Cpt, Cpi):
    """Build + compile the SPMD program for per-PAIR capacities (Cpt, Cpi)."""
    from concourse import bacc
    import concourse.tile as tile
    import concourse.mybir as mybir

    F32 = mybir.dt.float32
    F32R = mybir.dt.float32r
    BF16 = mybir.dt.bfloat16
    AF = mybir.ActivationFunctionType

    TP = _TPC // 2   # table pairs per core (4)
    IP = _IPC // 2   # index pairs per core (8)
    NTcols = TP * Cpt
    NIcols = IP * Cpi
    TCH = _TPC // _CPE   # table weight chunks (2)
    ICH = _IPC // _CPE   # index weight chunks (4)

    nc = bacc.Bacc(
        "TRN2",
        target_bir_lowering=False,
        debug=False,
        enable_asserts=False,
        num_devices=_N_CORES,
    )
    ft = nc.dram_tensor("ft", [_F, NTcols], BF16, kind="ExternalInput").ap()
    fi = nc.dram_tensor("fi", [_F, NIcols], BF16, kind="ExternalInput").ap()
    # host-packed, partition-major: [chunk, p, e_local*1024 + c*256 + g]
    wt = nc.dram_tensor("wt", [TCH, 128, _CPE * 4 * _G], BF16, kind="ExternalInput").ap()
    wi = nc.dram_tensor("wi", [ICH, 128, _CPE * 4 * _G], BF16, kind="ExternalInput").ap()
    bt = nc.dram_tensor("bt", [1, _TPC * _G], F32R, kind="ExternalInput").ap()
    bi = nc.dram_tensor("bi", [1, _IPC * _G], F32R, kind="ExternalInput").ap()
    wu = nc.dram_tensor("wu", [_F, _HID], BF16, kind="ExternalInput").ap()  # W_up.T
    buc = nc.dram_tensor("buc", [128, 4], F32, kind="ExternalInput").ap()  # b_up col-major
    on = nc.dram_tensor("on", [1, 512], F32R, kind="ExternalInput").ap()
    # outputs: per pair, both experts' logits for every sample in the segment
    ot = nc.dram_tensor("ot", [NTcols, 2 * _G], F32, kind="ExternalOutput").ap()
    oi = nc.dram_tensor("oi", [NIcols, 2 * _G], F32, kind="ExternalOutput").ap()

    otv = ot.rearrange("(j s) g -> j s g", s=Cpt)
    oiv = oi.rearrange("(j s) g -> j s g", s=Cpi)

    with tile.TileContext(nc) as tc:
        with (
            tc.tile_pool(name="persist", bufs=1) as persist,
            tc.tile_pool(name="wpool", bufs=6) as wpool,
            tc.tile_pool(name="opool", bufs=6) as opool,
            tc.tile_pool(name="ps1pool", bufs=3, space="PSUM") as ps1pool,
            tc.tile_pool(name="ps2pool", bufs=4, space="PSUM") as ps2pool,
        ):
            wu_sb = persist.tile([128, 2, _HID], BF16, name="wu_sb", tag="wu_sb")
            nc.sync.dma_start(out=wu_sb, in_=wu.rearrange("(c p) m -> p c m", p=128))
            buc_sb = persist.tile([128, 4], F32, name="buc_sb", tag="buc_sb")
            nc.scalar.dma_start(out=buc_sb, in_=buc)
            bt_sb = persist.tile([1, _TPC * _G], F32R, name="bt_sb", tag="bt_sb")
            nc.gpsimd.dma_start(out=bt_sb, in_=bt)
            bi_sb = persist.tile([1, _IPC * _G], F32R, name="bi_sb", tag="bi_sb")
            nc.gpsimd.dma_start(out=bi_sb, in_=bi)
            ones = persist.tile([1, 512], F32R, name="ones", tag="ones")
            nc.gpsimd.dma_start(out=ones, in_=on)

            # PE warmup: dense dummy matmuls during the fixed startup
            # window pre-ramp the HAM clock gate before real work arrives
            warm = persist.tile([128, 512], BF16, name="warm", tag="warm")
            nc.vector.memset(warm, 0.0)
            for _ in range(12):
                psw = ps1pool.tile([128, 512], F32, name="ps1", tag="ps1")
                nc.tensor.matmul(psw, lhsT=warm[:, :128], rhs=warm, start=True, stop=True)

            # feature loads + stage 1 for both roles first (fills PE early
            # while the first weight chunk streams in)
            f_sb = {}
            x_sb = {}
            for role, fdram, NC in (("t", ft, NTcols), ("i", fi, NIcols)):
                f_sb[role] = []
                feng = nc.sync if role == "t" else nc.scalar
                for c in range(2):
                    f_c = persist.tile(
                        [128, NC], BF16, name=f"f_{role}{c}", tag=f"f_{role}{c}"
                    )
                    feng.dma_start(out=f_c, in_=fdram[c * 128 : (c + 1) * 128, :])
                    f_sb[role].append(f_c)
                x_sb[role] = [
                    persist.tile(
                        [128, NC], BF16, name=f"x_{role}{m}", tag=f"x_{role}{m}"
                    )
                    for m in range(4)
                ]
            # interleave m-chunk pairs so consecutive matmuls hit different
            # PSUM banks (same-bank accumulation passes serialize the PE)
            for role, NC in (("t", NTcols), ("i", NIcols)):
                for n0 in range(0, NC, 512):
                    nw = min(512, NC - n0)
                    for m0 in (0, 2):
                        ps1s = {
                            m: ps1pool.tile([128, 512], F32, name="ps1", tag="ps1")
                            for m in (m0, m0 + 1)
                        }
                        for c in range(2):
                            for m in (m0, m0 + 1):
                                nc.tensor.matmul(
                                    ps1s[m][:, :nw],
                                    lhsT=wu_sb[:, c, m * 128 : (m + 1) * 128],
                                    rhs=f_sb[role][c][:, n0 : n0 + nw],
                                    start=(c == 0),
                                    stop=(c == 1),
                                )
                        for m in (m0, m0 + 1):
                            nc.scalar.activation(
                                out=x_sb[role][m][:, n0 : n0 + nw],
                                in_=ps1s[m][:, :nw],
                                func=AF.Relu,
                                bias=buc_sb[:, m : m + 1],
                            )

            # stage 2: weight chunks of 4 experts = 2 pairs. One pair => one
            # column segment, one PSUM bank, 4 fused K-chunk matmuls with
            # rhs spanning both experts (N=512), one sigmoid, one output DMA.
            for role, wdram, bsb, ov, nch, C in (
                ("t", wt, bt_sb, otv, TCH, Cpt),
                ("i", wi, bi_sb, oiv, ICH, Cpi),
            ):
                xs = x_sb[role]
                for ch in range(nch):
                    w_sb = wpool.tile(
                        [128, _CPE, 4, _G], BF16, name=f"w_sb_{role}", tag="w_sb"
                    )
                    eng = (nc.sync, nc.scalar, nc.gpsimd)[ch % 3]
                    eng.dma_start(
                        out=w_sb,
                        in_=wdram[ch].rearrange("p (e c g) -> p e c g", e=_CPE, c=4),
                    )
                    for s0 in range(0, C, 128):
                        sw = min(128, C - s0)
                        prs = list(range(_CPE // 2))
                        ps2s = {
                            pr: ps2pool.tile([128, 512], F32, name="ps2", tag="ps2")
                            for pr in prs
                        }
                        # bias first: spans the full zero region (one PSUM
                        # bank), initializing it for accumulation; pairs are
                        # interleaved so consecutive matmuls alternate banks
                        for pr in prs:
                            k0 = 2 * (ch * (_CPE // 2) + pr)
                            nc.tensor.matmul(
                                ps2s[pr][:sw, :],
                                lhsT=ones[:, :sw],
                                rhs=bsb[:, k0 * _G : (k0 + 2) * _G],
                                start=True,
                                stop=False,
                            )
                        for c in range(4):
                            for pr in prs:
                                j = ch * (_CPE // 2) + pr
                                nc.tensor.matmul(
                                    ps2s[pr][:sw, :],
                                    lhsT=xs[c][:, j * C + s0 : j * C + s0 + sw],
                                    rhs=w_sb[:, 2 * pr : 2 * pr + 2, c, :],
                                    start=False,
                                    stop=(c == 3),
                                )
                        for pr in prs:
                            j = ch * (_CPE // 2) + pr
                            o_sb = opool.tile(
                                [128, 2 * _G], F32, name="o_sb", tag="o_sb"
                            )
                            nc.scalar.activation(
                                out=o_sb[:sw], in_=ps2s[pr][:sw, :], func=AF.Sigmoid
                            )
                            oeng = nc.sync if (j % 2 == 0) else nc.scalar
                            oeng.dma_start(
                                out=ov[j][s0 : s0 + sw, :], in_=o_sb[:sw]
                            )

    nc.compile()
    return nc


def _get_nc(Cpt, Cpi):
    key = (Cpt, Cpi)
    if key not in _nc_cache:
        _nc_cache[key] = _build(Cpt, Cpi)
    return _nc_cache[key]


def _pack_weights(W, nexp):
    """[nexp, G, HID] -> [nexp/_CPE, 128, _CPE*4*G] partition-major chunks."""
    nch = nexp // _CPE
    A = W.reshape(nch, _CPE, _G, 4, 128)          # [ch, e, g, c, p]
    A = np.ascontiguousarray(A.transpose(0, 4, 1, 3, 2))  # [ch, p, e, c, g]
    return A.reshape(nch, 128, _CPE * 4 * _G).astype(ml_dtypes.bfloat16)


def _route(ids, mask, n_experts):
    """Per-PAIR sample lists: pair j owns experts 2j, 2j+1. Returns
    (pair_samples, pair_parity) lists of arrays."""
    samples, parity = [], []
    for j in range(n_experts // 2):
        s0 = np.flatnonzero((ids == 2 * j) & mask)
        s1 = np.flatnonzero((ids == 2 * j + 1) & mask)
        samples.append(np.concatenate([s0, s1]))
        parity.append(np.concatenate([np.zeros(len(s0), np.int64),
                                      np.ones(len(s1), np.int64)]))
    return samples, parity


def kernel(
    feature,
    table_ids,
    index_ids,
    table_mask,
    index_mask,
    W_up,
    b_up,
    W_table,
    b_table,
    W_index,
    b_index,
):
    global LAST_RESULTS
    from concourse.bass_utils import run_bass_kernel_spmd

    feature = np.ascontiguousarray(np.asarray(feature), dtype=np.float32)
    table_ids = np.asarray(table_ids).astype(np.int64)
    index_ids = np.asarray(index_ids).astype(np.int64)
    table_mask = np.asarray(table_mask).astype(bool)
    index_mask = np.asarray(index_mask).astype(bool)
    W_up = np.asarray(W_up, dtype=np.float32)
    b_up = np.asarray(b_up, dtype=np.float32)
    W_table = np.asarray(W_table, dtype=np.float32)
    b_table = np.asarray(b_table, dtype=np.float32)
    W_index = np.asarray(W_index, dtype=np.float32)
    b_index = np.asarray(b_index, dtype=np.float32)

    B = feature.shape[0]

    smp_t, par_t = _route(table_ids, table_mask, _N_TABLES)
    smp_i, par_i = _route(index_ids, index_mask, _N_INDEXES)
    # Uniform per-pair capacity so all 8 cores run one identical program.
    Cpt = max(8, -(-max(len(s) for s in smp_t) // 8) * 8)
    Cpi = max(8, -(-max(len(s) for s in smp_i) // 8) * 8)

    nc = _get_nc(Cpt, Cpi)

    TP = _TPC // 2
    IP = _IPC // 2
    W_upT = np.ascontiguousarray(W_up.T).astype(ml_dtypes.bfloat16)
    buc = np.ascontiguousarray(b_up.reshape(4, 128).T)
    ones = np.ones((1, 512), np.float32)

    in_maps = []
    for c in range(_N_CORES):
        ts = slice(c * _TPC, (c + 1) * _TPC)
        is_ = slice(c * _IPC, (c + 1) * _IPC)
        ft_c = np.zeros((_F, TP * Cpt), ml_dtypes.bfloat16)
        for j in range(TP):
            s = smp_t[c * TP + j]
            if len(s):
                ft_c[:, j * Cpt : j * Cpt + len(s)] = feature[s].T
        fi_c = np.zeros((_F, IP * Cpi), ml_dtypes.bfloat16)
        for j in range(IP):
            s = smp_i[c * IP + j]
            if len(s):
                fi_c[:, j * Cpi : j * Cpi + len(s)] = feature[s].T
        in_maps.append(
            {
                "ft": ft_c,
                "fi": fi_c,
                "wt": _pack_weights(W_table[ts], _TPC),
                "wi": _pack_weights(W_index[is_], _IPC),
                "bt": np.ascontiguousarray(b_table[ts].reshape(1, -1)),
                "bi": np.ascontiguousarray(b_index[is_].reshape(1, -1)),
                "wu": W_upT,
                "buc": buc,
                "on": ones,
            }
        )

    res = run_bass_kernel_spmd(
        nc, in_maps, core_ids=list(range(_N_CORES)), trace=TRACE
    )
    LAST_RESULTS = res

    out = np.zeros((2, B, _G), np.float32)
    for c in range(_N_CORES):
        rt = res.results[c]["ot"]
        ri = res.results[c]["oi"]
        for j in range(TP):
            s = smp_t[c * TP + j]
            if len(s):
                rows = rt[j * Cpt : j * Cpt + len(s)].reshape(len(s), 2, _G)
                out[0, s, :] = rows[np.arange(len(s)), par_t[c * TP + j], :]
        for j in range(IP):
            s = smp_i[c * IP + j]
            if len(s):
                rows = ri[j * Cpi : j * Cpi + len(s)].reshape(len(s), 2, _G)
                out[1, s, :] = rows[np.arange(len(s)), par_i[c * IP + j], :]
    return out


# revision 16
# speedup vs baseline: 1.0123x; 1.0123x over previous
"""Trainium2 Bass kernel for nn_CachePredictor (moe_routing).

Computation (see reference):
    x = relu(feature @ W_up.T + b_up)                      [B, 512]
    t_out = sigmoid(einsum('bf,bgf', x, W_table[tids]) + b_table[tids]) * tmask
    i_out = sigmoid(einsum('bf,bgf', x, W_index[iids]) + b_index[iids]) * imask
    out = stack([t_out, i_out])                            [2, B, 256]

Strategy: expert sharding. Per-sample gather of expert weights would move
~4 GB of HBM traffic; grouping samples by expert reads each expert matrix
exactly once (~96 MiB aggregate). Each of the 8 cores owns 8 table experts
and 16 index experts (~12.6 MiB of weights) and processes only the samples
routed to its experts. The host computes routing metadata (sample->expert
grouping, capacity padding) and arranges per-core inputs; all FLOPs (both
matmul stages, relu, sigmoid, bias adds) run on device.

Device program per core (single SPMD program; capacity-padded so all cores
share identical shapes):
  stage 1:  xT[512, nCols] = relu(W_upT.T @ featT + b_up)   PE + ACT(bias)
  stage 2:  per expert PAIR: out[rows, 2*256] = sigmoid(xT_seg.T @ [W_e0|W_e1].T + b)
Matmul operands are float32r (TF32-like 12-bit-mantissa streaming; full PE
rate at moving dim >= 256; ~2e-4 rel err). Experts are processed in PAIRS
sharing one column segment (the union of both experts' samples): every
sample in the segment is multiplied against BOTH experts' weights with one
N=512 moving pass, and the host keeps the valid half. This halves PE
instruction count (the per-instruction LDWEIGHTS + fixed overhead, ~350 ns,
dominates PE time otherwise) at the cost of cheap redundant FLOPs. The
pair's bias add is a K=1 ones-outer-product matmul issued FIRST so it
initializes the full PSUM zero region. Expert weights are host-packed
partition-major into 2 MiB chunks of 4 experts so each weight load is one
large fully-contiguous DMA (>=340 GB/s regime). Activations stay
transposed ([feature, sample]) end to end.

Masked-off samples are never routed (reference zeroes them); the host
scatters computed rows back and leaves the rest zero.
"""

import ml_dtypes
import numpy as np

_N_CORES = 8
_F = 256        # feature dim
_HID = 512      # up-projection width
_G = 256        # buckets
_N_TABLES = 64
_N_INDEXES = 128
_TPC = _N_TABLES // _N_CORES    # table experts per core
_IPC = _N_INDEXES // _N_CORES   # index experts per core
_CPE = 8                        # experts per weight chunk (2 MiB bf16)
_TPAIRS = _N_TABLES // 2        # global table pairs
_IPAIRS = _N_INDEXES // 2       # global index pairs

_nc_cache = {}

# Set by a test harness to capture HW profiles; harmless when unused.
TRACE = False
LAST_RESULTS = None


def _build(Cpt, Cpi):
    """Build + compile the SPMD program for per-PAIR capacities (Cpt, Cpi)."""
    from concourse import bacc
    import concourse.tile as tile
    import concourse.mybir as mybir

    F32 = mybir.dt.float32
    F32R = mybir.dt.float32r
    BF16 = mybir.dt.bfloat16
    AF = mybir.ActivationFunctionType

    TP = _TPC // 2   # table pairs per core (4)
    IP = _IPC // 2   # index pairs per core (8)
    NTcols = TP * Cpt
    NIcols = IP * Cpi
    TCH = _TPC // _CPE   # table weight chunks (2)
    ICH = _IPC // _CPE   # index weight chunks (4)

    nc = bacc.Bacc(
        "TRN2",
        target_bir_lowering=False,
        debug=False,
        enable_asserts=False,
        num_devices=_N_CORES,
    )
    ft = nc.dram_tensor("ft", [_F, NTcols], BF16, kind="ExternalInput").ap()
    fi = nc.dram_tensor("fi", [_F, NIcols], BF16, kind="ExternalInput").ap()
    # host-packed, partition-major: [chunk, p, e_local*1024 + c*256 + g]
    wt = nc.dram_tensor("wt", [TCH, 128, _CPE * 4 * _G], BF16, kind="ExternalInput").ap()
    wi = nc.dram_tensor("wi", [ICH, 128, _CPE * 4 * _G], BF16, kind="ExternalInput").ap()
    bt = nc.dram_tensor("bt", [1, _TPC * _G], F32R, kind="ExternalInput").ap()
    bi = nc.dram_tensor("bi", [1, _IPC * _G], F32R, kind="ExternalInput").ap()
    wu = nc.dram_tensor("wu", [_F, _HID], BF16, kind="ExternalInput").ap()  # W_up.T
    buc = nc.dram_tensor("buc", [128, 4], F32, kind="ExternalInput").ap()  # b_up col-major
    on = nc.dram_tensor("on", [1, 512], F32R, kind="ExternalInput").ap()
    # outputs: per pair, both experts' logits for every sample in the segment
    ot = nc.dram_tensor("ot", [NTcols, 2 * _G], F32, kind="ExternalOutput").ap()
    oi = nc.dram_tensor("oi", [NIcols, 2 * _G], F32, kind="ExternalOutput").ap()

    otv = ot.rearrange("(j s) g -> j s g", s=Cpt)
    oiv = oi.rearrange("(j s) g -> j s g", s=Cpi)

    with tile.TileContext(nc) as tc:
        with (
            tc.tile_pool(name="persist", bufs=1) as persist,
            tc.tile_pool(name="wpool", bufs=6) as wpool,
            tc.tile_pool(name="opool", bufs=6) as opool,
            tc.tile_pool(name="ps1pool", bufs=3, space="PSUM") as ps1pool,
            tc.tile_pool(name="ps2pool", bufs=4, space="PSUM") as ps2pool,
        ):
            wu_sb = persist.tile([128, 2, _HID], BF16, name="wu_sb", tag="wu_sb")
            nc.sync.dma_start(out=wu_sb, in_=wu.rearrange("(c p) m -> p c m", p=128))
            buc_sb = persist.tile([128, 4], F32, name="buc_sb", tag="buc_sb")
            nc.scalar.dma_start(out=buc_sb, in_=buc)
            bt_sb = persist.tile([1, _TPC * _G], F32R, name="bt_sb", tag="bt_sb")
            nc.gpsimd.dma_start(out=bt_sb, in_=bt)
            bi_sb = persist.tile([1, _IPC * _G], F32R, name="bi_sb", tag="bi_sb")
            nc.gpsimd.dma_start(out=bi_sb, in_=bi)
            ones = persist.tile([1, 512], F32R, name="ones", tag="ones")
            nc.gpsimd.dma_start(out=ones, in_=on)

            # PE warmup: dense dummy matmuls during the fixed startup
            # window pre-ramp the HAM clock gate before real work arrives
            warm = persist.tile([128, 512], BF16, name="warm", tag="warm")
            nc.vector.memset(warm, 0.0)
            for _ in range(12):
                psw = ps1pool.tile([128, 512], F32, name="ps1", tag="ps1")
                nc.tensor.matmul(psw, lhsT=warm[:, :128], rhs=warm, start=True, stop=True)

            # feature loads + stage 1 for both roles first (fills PE early
            # while the first weight chunk streams in)
            f_sb = {}
            x_sb = {}
            for role, fdram, NC in (("t", ft, NTcols), ("i", fi, NIcols)):
                f_sb[role] = []
                feng = nc.sync if role == "t" else nc.scalar
                for c in range(2):
                    f_c = persist.tile(
                        [128, NC], BF16, name=f"f_{role}{c}", tag=f"f_{role}{c}"
                    )
                    feng.dma_start(out=f_c, in_=fdram[c * 128 : (c + 1) * 128, :])
                    f_sb[role].append(f_c)
                x_sb[role] = [
                    persist.tile(
                        [128, NC], BF16, name=f"x_{role}{m}", tag=f"x_{role}{m}"
                    )
                    for m in range(4)
                ]
            # interleave m-chunk pairs so consecutive matmuls hit different
            # PSUM banks (same-bank accumulation passes serialize the PE)
            for role, NC in (("t", NTcols), ("i", NIcols)):
                for n0 in range(0, NC, 512):
                    nw = min(512, NC - n0)
                    for m0 in (0, 2):
                        ps1s = {
                            m: ps1pool.tile([128, 512], F32, name="ps1", tag="ps1")
                            for m in (m0, m0 + 1)
                        }
                        for c in range(2):
                            for m in (m0, m0 + 1):
                                nc.tensor.matmul(
                                    ps1s[m][:, :nw],
                                    lhsT=wu_sb[:, c, m * 128 : (m + 1) * 128],
                                    rhs=f_sb[role][c][:, n0 : n0 + nw],
                                    start=(c == 0),
                                    stop=(c == 1),
                                )
                        for m in (m0, m0 + 1):
                            nc.scalar.activation(
                                out=x_sb[role][m][:, n0 : n0 + nw],
                                in_=ps1s[m][:, :nw],
                                func=AF.Relu,
                                bias=buc_sb[:, m : m + 1],
                            )

            # stage 2: weight chunks of 4 experts = 2 pairs. One pair => one
            # column segment, one PSUM bank, 4 fused K-chunk matmuls with
            # rhs spanning both experts (N=512), one sigmoid, one output DMA.
            for role, wdram, bsb, ov, nch, C in (
                ("t", wt, bt_sb, otv, TCH, Cpt),
                ("i", wi, bi_sb, oiv, ICH, Cpi),
            ):
                xs = x_sb[role]
                for ch in range(nch):
                    w_sb = wpool.tile(
                        [128, _CPE, 4, _G], BF16, name=f"w_sb_{role}", tag="w_sb"
                    )
                    # each chunk split across both HWDGE rings so chunks
                    # complete in consumption order at combined rate
                    wv = wdram[ch].rearrange("p (e c g) -> p e c g", e=_CPE, c=4)
                    h = _CPE // 2
                    nc.sync.dma_start(out=w_sb[:, :h], in_=wv[:, :h])
                    nc.scalar.dma_start(out=w_sb[:, h:], in_=wv[:, h:])
                    for s0 in range(0, C, 128):
                        sw = min(128, C - s0)
                        prs = list(range(_CPE // 2))
                        ps2s = {
                            pr: ps2pool.tile([128, 512], F32, name="ps2", tag="ps2")
                            for pr in prs
                        }
                        # bias first: spans the full zero region (one PSUM
                        # bank), initializing it for accumulation; pairs are
                        # interleaved so consecutive matmuls alternate banks
                        for pr in prs:
                            k0 = 2 * (ch * (_CPE // 2) + pr)
                            nc.tensor.matmul(
                                ps2s[pr][:sw, :],
                                lhsT=ones[:, :sw],
                                rhs=bsb[:, k0 * _G : (k0 + 2) * _G],
                                start=True,
                                stop=False,
                            )
                        for c in range(4):
                            for pr in prs:
                                j = ch * (_CPE // 2) + pr
                                nc.tensor.matmul(
                                    ps2s[pr][:sw, :],
                                    lhsT=xs[c][:, j * C + s0 : j * C + s0 + sw],
                                    rhs=w_sb[:, 2 * pr : 2 * pr + 2, c, :],
                                    start=False,
                                    stop=(c == 3),
                                )
                        for pr in prs:
                            j = ch * (_CPE // 2) + pr
                            o_sb = opool.tile(
                                [128, 2 * _G], F32, name="o_sb", tag="o_sb"
                            )
                            nc.scalar.activation(
                                out=o_sb[:sw], in_=ps2s[pr][:sw, :], func=AF.Sigmoid
                            )
                            oeng = nc.sync if (j % 2 == 0) else nc.scalar
                            oeng.dma_start(
                                out=ov[j][s0 : s0 + sw, :], in_=o_sb[:sw]
                            )

    nc.compile()
    return nc


def _get_nc(Cpt, Cpi):
    key = (Cpt, Cpi)
    if key not in _nc_cache:
        _nc_cache[key] = _build(Cpt, Cpi)
    return _nc_cache[key]


def _pack_weights(W, nexp):
    """[nexp, G, HID] -> [nexp/_CPE, 128, _CPE*4*G] partition-major chunks."""
    nch = nexp // _CPE
    A = W.reshape(nch, _CPE, _G, 4, 128)          # [ch, e, g, c, p]
    A = np.ascontiguousarray(A.transpose(0, 4, 1, 3, 2))  # [ch, p, e, c, g]
    return A.reshape(nch, 128, _CPE * 4 * _G).astype(ml_dtypes.bfloat16)


def _route(ids, mask, n_experts):
    """Per-PAIR sample lists: pair j owns experts 2j, 2j+1. Returns
    (pair_samples, pair_parity) lists of arrays."""
    samples, parity = [], []
    for j in range(n_experts // 2):
        s0 = np.flatnonzero((ids == 2 * j) & mask)
        s1 = np.flatnonzero((ids == 2 * j + 1) & mask)
        samples.append(np.concatenate([s0, s1]))
        parity.append(np.concatenate([np.zeros(len(s0), np.int64),
                                      np.ones(len(s1), np.int64)]))
    return samples, parity


def kernel(
    feature,
    table_ids,
    index_ids,
    table_mask,
    index_mask,
    W_up,
    b_up,
    W_table,
    b_table,
    W_index,
    b_index,
):
    global LAST_RESULTS
    from concourse.bass_utils import run_bass_kernel_spmd

    feature = np.ascontiguousarray(np.asarray(feature), dtype=np.float32)
    table_ids = np.asarray(table_ids).astype(np.int64)
    index_ids = np.asarray(index_ids).astype(np.int64)
    table_mask = np.asarray(table_mask).astype(bool)
    index_mask = np.asarray(index_mask).astype(bool)
    W_up = np.asarray(W_up, dtype=np.float32)
    b_up = np.asarray(b_up, dtype=np.float32)
    W_table = np.asarray(W_table, dtype=np.float32)
    b_table = np.asarray(b_table, dtype=np.float32)
    W_index = np.asarray(W_index, dtype=np.float32)
    b_index = np.asarray(b_index, dtype=np.float32)

    B = feature.shape[0]

    smp_t, par_t = _route(table_ids, table_mask, _N_TABLES)
    smp_i, par_i = _route(index_ids, index_mask, _N_INDEXES)
    # Uniform per-pair capacity so all 8 cores run one identical program.
    Cpt = max(8, -(-max(len(s) for s in smp_t) // 8) * 8)
    Cpi = max(8, -(-max(len(s) for s in smp_i) // 8) * 8)

    nc = _get_nc(Cpt, Cpi)

    TP = _TPC // 2
    IP = _IPC // 2
    W_upT = np.ascontiguousarray(W_up.T).astype(ml_dtypes.bfloat16)
    buc = np.ascontiguousarray(b_up.reshape(4, 128).T)
    ones = np.ones((1, 512), np.float32)

    in_maps = []
    for c in range(_N_CORES):
        ts = slice(c * _TPC, (c + 1) * _TPC)
        is_ = slice(c * _IPC, (c + 1) * _IPC)
        ft_c = np.zeros((_F, TP * Cpt), ml_dtypes.bfloat16)
        for j in range(TP):
            s = smp_t[c * TP + j]
            if len(s):
                ft_c[:, j * Cpt : j * Cpt + len(s)] = feature[s].T
        fi_c = np.zeros((_F, IP * Cpi), ml_dtypes.bfloat16)
        for j in range(IP):
            s = smp_i[c * IP + j]
            if len(s):
                fi_c[:, j * Cpi : j * Cpi + len(s)] = feature[s].T
        in_maps.append(
            {
                "ft": ft_c,
                "fi": fi_c,
                "wt": _pack_weights(W_table[ts], _TPC),
                "wi": _pack_weights(W_index[is_], _IPC),
                "bt": np.ascontiguousarray(b_table[ts].reshape(1, -1)),
                "bi": np.ascontiguousarray(b_index[is_].reshape(1, -1)),
                "wu": W_upT,
                "buc": buc,
                "on": ones,
            }
        )

    res = run_bass_kernel_spmd(
        nc, in_maps, core_ids=list(range(_N_CORES)), trace=TRACE
    )
    LAST_RESULTS = res

    out = np.zeros((2, B, _G), np.float32)
    for c in range(_N_CORES):
        rt = res.results[c]["ot"]
        ri = res.results[c]["oi"]
        for j in range(TP):
            s = smp_t[c * TP + j]
            if len(s):
                rows = rt[j * Cpt : j * Cpt + len(s)].reshape(len(s), 2, _G)
                out[0, s, :] = rows[np.arange(len(s)), par_t[c * TP + j], :]
        for j in range(IP):
            s = smp_i[c * IP + j]
            if len(s):
                rows = ri[j * Cpi : j * Cpi + len(s)].reshape(len(s), 2, _G)
                out[1, s, :] = rows[np.arange(len(s)), par_i[c * IP + j], :]
    return out


# revision 18
# speedup vs baseline: 1.0827x; 1.0695x over previous
"""Trainium2 Bass kernel for nn_CachePredictor (moe_routing).

Computation (see reference):
    x = relu(feature @ W_up.T + b_up)                      [B, 512]
    t_out = sigmoid(einsum('bf,bgf', x, W_table[tids]) + b_table[tids]) * tmask
    i_out = sigmoid(einsum('bf,bgf', x, W_index[iids]) + b_index[iids]) * imask
    out = stack([t_out, i_out])                            [2, B, 256]

Strategy: expert sharding. Per-sample gather of expert weights would move
~4 GB of HBM traffic; grouping samples by expert reads each expert matrix
exactly once (~96 MiB aggregate). Each of the 8 cores owns 8 table experts
and 16 index experts (~12.6 MiB of weights) and processes only the samples
routed to its experts. The host computes routing metadata (sample->expert
grouping, capacity padding) and arranges per-core inputs; all FLOPs (both
matmul stages, relu, sigmoid, bias adds) run on device.

Device program per core (single SPMD program; capacity-padded so all cores
share identical shapes):
  stage 1:  xT[512, nCols] = relu(W_upT.T @ featT + b_up)   PE + ACT(bias)
  stage 2:  per expert PAIR: out[rows, 2*256] = sigmoid(xT_seg.T @ [W_e0|W_e1].T + b)
Matmul operands are float32r (TF32-like 12-bit-mantissa streaming; full PE
rate at moving dim >= 256; ~2e-4 rel err). Experts are processed in PAIRS
sharing one column segment (the union of both experts' samples): every
sample in the segment is multiplied against BOTH experts' weights with one
N=512 moving pass, and the host keeps the valid half. This halves PE
instruction count (the per-instruction LDWEIGHTS + fixed overhead, ~350 ns,
dominates PE time otherwise) at the cost of cheap redundant FLOPs. The
pair's bias add is a K=1 ones-outer-product matmul issued FIRST so it
initializes the full PSUM zero region. Expert weights are host-packed
partition-major into 2 MiB chunks of 4 experts so each weight load is one
large fully-contiguous DMA (>=340 GB/s regime). Activations stay
transposed ([feature, sample]) end to end.

Masked-off samples are never routed (reference zeroes them); the host
scatters computed rows back and leaves the rest zero.
"""

import ml_dtypes
import numpy as np

_N_CORES = 8
_F = 256        # feature dim
_HID = 512      # up-projection width
_G = 256        # buckets
_N_TABLES = 64
_N_INDEXES = 128
_TPC = _N_TABLES // _N_CORES    # table experts per core
_IPC = _N_INDEXES // _N_CORES   # index experts per core
_CPE = 4                        # experts per weight chunk (1 MiB bf16)
_TPAIRS = _N_TABLES // 2        # global table pairs
_IPAIRS = _N_INDEXES // 2       # global index pairs

_nc_cache = {}

# Set by a test harness to capture HW profiles; harmless when unused.
TRACE = False
LAST_RESULTS = None


def _build(Cpt, Cpi):
    """Build + compile the SPMD program for per-PAIR capacities (Cpt, Cpi)."""
    from concourse import bacc
    import concourse.tile as tile
    import concourse.mybir as mybir

    F32 = mybir.dt.float32
    F32R = mybir.dt.float32r
    BF16 = mybir.dt.bfloat16
    AF = mybir.ActivationFunctionType

    TP = _TPC // 2   # table pairs per core (4)
    IP = _IPC // 2   # index pairs per core (8)
    NTcols = TP * Cpt
    NIcols = IP * Cpi
    TCH = _TPC // _CPE   # table weight chunks (2)
    ICH = _IPC // _CPE   # index weight chunks (4)

    nc = bacc.Bacc(
        "TRN2",
        target_bir_lowering=False,
        debug=False,
        enable_asserts=False,
        num_devices=_N_CORES,
    )
    fa = nc.dram_tensor("fa", [_F, NTcols + NIcols], BF16, kind="ExternalInput").ap()
    # host-packed, partition-major: [chunk, p, e_local*1024 + c*256 + g]
    wt = nc.dram_tensor("wt", [TCH, 128, _CPE * 4 * _G], BF16, kind="ExternalInput").ap()
    wi = nc.dram_tensor("wi", [ICH, 128, _CPE * 4 * _G], BF16, kind="ExternalInput").ap()
    bt = nc.dram_tensor("bt", [1, _TPC * _G], F32R, kind="ExternalInput").ap()
    bi = nc.dram_tensor("bi", [1, _IPC * _G], F32R, kind="ExternalInput").ap()
    wu = nc.dram_tensor("wu", [_F, _HID], BF16, kind="ExternalInput").ap()  # W_up.T
    buc = nc.dram_tensor("buc", [128, 4], F32, kind="ExternalInput").ap()  # b_up col-major
    on = nc.dram_tensor("on", [1, 512], F32R, kind="ExternalInput").ap()
    # outputs: per pair, both experts' logits for every sample in the segment
    ot = nc.dram_tensor("ot", [NTcols, 2 * _G], F32, kind="ExternalOutput").ap()
    oi = nc.dram_tensor("oi", [NIcols, 2 * _G], F32, kind="ExternalOutput").ap()

    otv = ot.rearrange("(j s) g -> s j g", s=Cpt)
    oiv = oi.rearrange("(j s) g -> s j g", s=Cpi)

    with tile.TileContext(nc) as tc:
        with (
            tc.tile_pool(name="persist", bufs=1) as persist,
            tc.tile_pool(name="wpool", bufs=6) as wpool,
            tc.tile_pool(name="opool", bufs=6) as opool,
            tc.tile_pool(name="ps1pool", bufs=3, space="PSUM") as ps1pool,
            tc.tile_pool(name="ps2pool", bufs=4, space="PSUM") as ps2pool,
        ):
            wu_sb = persist.tile([128, 2, _HID], BF16, name="wu_sb", tag="wu_sb")
            nc.sync.dma_start(out=wu_sb, in_=wu.rearrange("(c p) m -> p c m", p=128))
            buc_sb = persist.tile([128, 4], F32, name="buc_sb", tag="buc_sb")
            nc.scalar.dma_start(out=buc_sb, in_=buc)
            bt_sb = persist.tile([1, _TPC * _G], F32R, name="bt_sb", tag="bt_sb")
            nc.gpsimd.dma_start(out=bt_sb, in_=bt)
            bi_sb = persist.tile([1, _IPC * _G], F32R, name="bi_sb", tag="bi_sb")
            nc.gpsimd.dma_start(out=bi_sb, in_=bi)
            ones = persist.tile([1, 512], F32R, name="ones", tag="ones")
            nc.gpsimd.dma_start(out=ones, in_=on)

            # PE warmup: dense dummy matmuls during the fixed startup
            # window pre-ramp the HAM clock gate before real work arrives
            warm = persist.tile([128, 512], BF16, name="warm", tag="warm")
            nc.vector.memset(warm, 0.0)
            for _ in range(12):
                psw = ps1pool.tile([128, 512], F32, name="ps1", tag="ps1")
                nc.tensor.matmul(psw, lhsT=warm[:, :128], rhs=warm, start=True, stop=True)

            # feature loads + stage 1 for both roles first (fills PE early
            # while the first weight chunk streams in)
            NA = NTcols + NIcols
            f_sb = []
            for c in range(2):
                f_c = persist.tile([128, NA], BF16, name=f"f_a{c}", tag=f"f_a{c}")
                feng = nc.sync if c == 0 else nc.scalar
                feng.dma_start(out=f_c, in_=fa[c * 128 : (c + 1) * 128, :])
                f_sb.append(f_c)
            x_sb = {}
            off = {"t": 0, "i": NTcols}
            for role, NC in (("t", NTcols), ("i", NIcols)):
                x_sb[role] = [
                    persist.tile(
                        [128, NC], BF16, name=f"x_{role}{m}", tag=f"x_{role}{m}"
                    )
                    for m in range(4)
                ]
            # interleave m-chunk pairs so consecutive matmuls hit different
            # PSUM banks (same-bank accumulation passes serialize the PE)
            for role, NC in (("t", NTcols), ("i", NIcols)):
                for n0 in range(0, NC, 512):
                    nw = min(512, NC - n0)
                    for m0 in (0, 2):
                        ps1s = {
                            m: ps1pool.tile([128, 512], F32, name="ps1", tag="ps1")
                            for m in (m0, m0 + 1)
                        }
                        for c in range(2):
                            for m in (m0, m0 + 1):
                                nc.tensor.matmul(
                                    ps1s[m][:, :nw],
                                    lhsT=wu_sb[:, c, m * 128 : (m + 1) * 128],
                                    rhs=f_sb[c][:, off[role] + n0 : off[role] + n0 + nw],
                                    start=(c == 0),
                                    stop=(c == 1),
                                )
                        for m in (m0, m0 + 1):
                            nc.scalar.activation(
                                out=x_sb[role][m][:, n0 : n0 + nw],
                                in_=ps1s[m][:, :nw],
                                func=AF.Relu,
                                bias=buc_sb[:, m : m + 1],
                            )

            # stage 2: weight chunks of 4 experts = 2 pairs. One pair => one
            # column segment, one PSUM bank, 4 fused K-chunk matmuls with
            # rhs spanning both experts (N=512), one sigmoid, one output DMA.
            for role, wdram, bsb, ov, nch, C in (
                ("t", wt, bt_sb, otv, TCH, Cpt),
                ("i", wi, bi_sb, oiv, ICH, Cpi),
            ):
                xs = x_sb[role]
                for ch in range(nch):
                    w_sb = wpool.tile(
                        [128, _CPE, 4, _G], BF16, name=f"w_sb_{role}", tag="w_sb"
                    )
                    eng = nc.scalar if (ch % 2 == 0) else nc.sync
                    eng.dma_start(
                        out=w_sb,
                        in_=wdram[ch].rearrange("p (e c g) -> p e c g", e=_CPE, c=4),
                    )
                    for s0 in range(0, C, 128):
                        sw = min(128, C - s0)
                        prs = list(range(_CPE // 2))
                        ps2s = {
                            pr: ps2pool.tile([128, 512], F32, name="ps2", tag="ps2")
                            for pr in prs
                        }
                        # bias first: spans the full zero region (one PSUM
                        # bank), initializing it for accumulation; pairs are
                        # interleaved so consecutive matmuls alternate banks
                        for pr in prs:
                            k0 = 2 * (ch * (_CPE // 2) + pr)
                            nc.tensor.matmul(
                                ps2s[pr][:sw, :],
                                lhsT=ones[:, :sw],
                                rhs=bsb[:, k0 * _G : (k0 + 2) * _G],
                                start=True,
                                stop=False,
                            )
                        for c in range(4):
                            for pr in prs:
                                j = ch * (_CPE // 2) + pr
                                nc.tensor.matmul(
                                    ps2s[pr][:sw, :],
                                    lhsT=xs[c][:, j * C + s0 : j * C + s0 + sw],
                                    rhs=w_sb[:, 2 * pr : 2 * pr + 2, c, :],
                                    start=False,
                                    stop=(c == 3),
                                )
                        for pr in prs:
                            j = ch * len(prs) + pr
                            o_sb = opool.tile(
                                [128, 2 * _G], F32, name="o_sb", tag="o_sb"
                            )
                            nc.scalar.activation(
                                out=o_sb[:sw], in_=ps2s[pr][:sw, :], func=AF.Sigmoid
                            )
                            oeng = nc.sync if (j % 2 == 0) else nc.scalar
                            oeng.dma_start(
                                out=ov[s0 : s0 + sw, j : j + 1, :], in_=o_sb[:sw]
                            )

    nc.compile()
    return nc


def _get_nc(Cpt, Cpi):
    key = (Cpt, Cpi)
    if key not in _nc_cache:
        _nc_cache[key] = _build(Cpt, Cpi)
    return _nc_cache[key]


def _pack_weights(W, nexp):
    """[nexp, G, HID] -> [nexp/_CPE, 128, _CPE*4*G] partition-major chunks."""
    nch = nexp // _CPE
    A = W.reshape(nch, _CPE, _G, 4, 128)          # [ch, e, g, c, p]
    A = np.ascontiguousarray(A.transpose(0, 4, 1, 3, 2))  # [ch, p, e, c, g]
    return A.reshape(nch, 128, _CPE * 4 * _G).astype(ml_dtypes.bfloat16)


def _route(ids, mask, n_experts):
    """Per-PAIR sample lists: pair j owns experts 2j, 2j+1. Returns
    (pair_samples, pair_parity) lists of arrays."""
    samples, parity = [], []
    for j in range(n_experts // 2):
        s0 = np.flatnonzero((ids == 2 * j) & mask)
        s1 = np.flatnonzero((ids == 2 * j + 1) & mask)
        samples.append(np.concatenate([s0, s1]))
        parity.append(np.concatenate([np.zeros(len(s0), np.int64),
                                      np.ones(len(s1), np.int64)]))
    return samples, parity


def kernel(
    feature,
    table_ids,
    index_ids,
    table_mask,
    index_mask,
    W_up,
    b_up,
    W_table,
    b_table,
    W_index,
    b_index,
):
    global LAST_RESULTS
    from concourse.bass_utils import run_bass_kernel_spmd

    feature = np.ascontiguousarray(np.asarray(feature), dtype=np.float32)
    table_ids = np.asarray(table_ids).astype(np.int64)
    index_ids = np.asarray(index_ids).astype(np.int64)
    table_mask = np.asarray(table_mask).astype(bool)
    index_mask = np.asarray(index_mask).astype(bool)
    W_up = np.asarray(W_up, dtype=np.float32)
    b_up = np.asarray(b_up, dtype=np.float32)
    W_table = np.asarray(W_table, dtype=np.float32)
    b_table = np.asarray(b_table, dtype=np.float32)
    W_index = np.asarray(W_index, dtype=np.float32)
    b_index = np.asarray(b_index, dtype=np.float32)

    B = feature.shape[0]

    smp_t, par_t = _route(table_ids, table_mask, _N_TABLES)
    smp_i, par_i = _route(index_ids, index_mask, _N_INDEXES)
    # Uniform per-pair capacity so all 8 cores run one identical program.
    Cpt = max(8, -(-max(len(s) for s in smp_t) // 8) * 8)
    Cpi = max(8, -(-max(len(s) for s in smp_i) // 8) * 8)

    nc = _get_nc(Cpt, Cpi)

    TP = _TPC // 2
    IP = _IPC // 2
    W_upT = np.ascontiguousarray(W_up.T).astype(ml_dtypes.bfloat16)
    buc = np.ascontiguousarray(b_up.reshape(4, 128).T)
    ones = np.ones((1, 512), np.float32)

    in_maps = []
    for c in range(_N_CORES):
        ts = slice(c * _TPC, (c + 1) * _TPC)
        is_ = slice(c * _IPC, (c + 1) * _IPC)
        fa_c = np.zeros((_F, TP * Cpt + IP * Cpi), ml_dtypes.bfloat16)
        ft_c = fa_c[:, : TP * Cpt]
        for j in range(TP):
            s = smp_t[c * TP + j]
            if len(s):
                ft_c[:, j * Cpt : j * Cpt + len(s)] = feature[s].T
        fi_c = fa_c[:, TP * Cpt :]
        for j in range(IP):
            s = smp_i[c * IP + j]
            if len(s):
                fi_c[:, j * Cpi : j * Cpi + len(s)] = feature[s].T
        in_maps.append(
            {
                "fa": fa_c,
                "wt": _pack_weights(W_table[ts], _TPC),
                "wi": _pack_weights(W_index[is_], _IPC),
                "bt": np.ascontiguousarray(b_table[ts].reshape(1, -1)),
                "bi": np.ascontiguousarray(b_index[is_].reshape(1, -1)),
                "wu": W_upT,
                "buc": buc,
                "on": ones,
            }
        )

    res = run_bass_kernel_spmd(
        nc, in_maps, core_ids=list(range(_N_CORES)), trace=TRACE
    )
    LAST_RESULTS = res

    out = np.zeros((2, B, _G), np.float32)
    for c in range(_N_CORES):
        rt = res.results[c]["ot"]
        ri = res.results[c]["oi"]
        for j in range(TP):
            s = smp_t[c * TP + j]
            if len(s):
                rows = rt[j * Cpt : j * Cpt + len(s)].reshape(len(s), 2, _G)
                out[0, s, :] = rows[np.arange(len(s)), par_t[c * TP + j], :]
        for j in range(IP):
            s = smp_i[c * IP + j]
            if len(s):
                rows = ri[j * Cpi : j * Cpi + len(s)].reshape(len(s), 2, _G)
                out[1, s, :] = rows[np.arange(len(s)), par_i[c * IP + j], :]
    return out
